# revision 39
# baseline (speedup 1.0000x reference)
"""Bass/Tile fused attention kernel for nn_AttentionLayer (B=4, S=4096, 256->64).

Sharding: 8 cores = 4 batches x 2 query-halves. Each core gets xT = x[b].T
(bf16, host-transposed, rolled so its own 2048 queries are keys 0..2047),
computes q/k/v projections + flash attention fully on-chip, and writes its
[2048, 64] output slice.

Inputs per core (host-packed for DMA efficiency):
  xT    [128, 4, 2, 1024] bf16  x^T in n-quarters, per-partition contiguous
                                (c, n) runs; pulled by 3 DMA rings (sync,
                                gpsimd SWDGE, scalar) so the scalar/ACT
                                sequencer's descriptor-gen list stays short
  wpack [128, 451]        bf16  Wq|Wk|Wv ([128, 2, 64] c-tile-major) + bq2 +
                                bk2 + R matrix, one transfer

On-chip layouts:
  qT2_sb [128, M]      bf16   q^T duplicated on partition halves (row-packed QK)
  kT2_sb [128, S/2]    bf16   k^T packed: parts 0:64 = even n-tiles, 64:128 = odd
  v_sb   [128, NT*65]  bf16   v natural per n-tile + ones column (AV stationary)

Startup: a dummy exp hoists ACT_TABLE_LOAD ahead of the scalar-issued DMA
descriptor-gens; PE warmup matmuls open the HAM clock gate during the x
transfer.  Projections run with 3-buf psum pools (only 7 banks held) so the
attention pools land on early-freed banks.

Attention (per 512-query chunk): for each n-tile pair j: two row-packed
K=64 matmuls -> scores^T [128, 1024] psum; exp alternates between ACT
(exp(s/8) -> bf16) and DVE (single-pass Schraudolph fast-exp -> i16 tile
whose bf16 bitcast feeds AV directly); two AV matmuls accumulate [65, 512]
psum (row 64 = softmax denominator l).  q quarters 1-3 are emitted inside
chunk 0's pair stream, borrowing s-ring psum slots.
Epilogue: PE "transpose" by R = [[I_64, 0], [bv^T, 1]] folds the bv bias in,
taking stride-4 ob slices so pt partition p = query mlo+4p+t and each
out-DMA descriptor covers 4 contiguous 64-col rows (1KB); DVE reciprocal +
scale -> z.  The last chunk's transposes borrow s-ring slots (free by then)
instead of the single-buf pt pool.
"""

from contextlib import ExitStack

import numpy as np
import concourse.bass as bass
import concourse.mybir as mybir
import concourse.tile as tile
from concourse.masks import make_identity

BF16 = mybir.dt.bfloat16
F32 = mybir.dt.float32
AF = mybir.ActivationFunctionType


MAX_WAITS = 1  # this image's walrus allows a single sem wait on most instructions
AV_SPLIT = False  # K-split AV regressed on HW (no row-tile stream overlap)


def _max_waits(inst):
    return MAX_WAITS


def split_excess_waits(nc):
    """Move excess sem-waits from any instruction onto same-engine NOPs
    inserted immediately before it (walrus wait-slot limit workaround)."""
    for f in nc.m.functions:
        for bb in f.blocks:
            insts = list(bb.instructions)
            out, n_new = [], 0
            for inst in insts:
                mw = _max_waits(inst)
                si = inst.sync_info
                waits = list(si.on_wait) if si and si.on_wait else []
                if len(waits) > mw:
                    excess = waits[: len(waits) - mw]
                    keep = waits[len(waits) - mw :]
                    for i in range(0, len(excess), MAX_WAITS):
                        nop = mybir.InstNoOp(
                            name=f"{inst.name}-wsplit{i}", ins=[], outs=[]
                        )
                        nop.engine = inst.engine
                        nop.sync_info = mybir.SyncInfo(
                            on_wait=excess[i : i + MAX_WAITS], on_update=[]
                        )
                        nc.register_instruction(nop, overwrite=True)
                        out.append(nop)
                        n_new += 1
                    inst.sync_info = mybir.SyncInfo(
                        on_wait=keep, on_update=si.on_update
                    )
                out.append(inst)
            if n_new:
                bb.instructions = out


def _ldw_sig(ap_str, tile_position, perf_mode, is_transpose):
    return (ap_str, tile_position, perf_mode, is_transpose)


def _ldw_rect(inst, w):
    tp = inst.tile_position or (0, 0)
    rows = w.ap[0][1]
    cols = 1
    for d in list(w.ap)[1:]:
        cols *= d[1]
    return (tp[0], tp[1], rows, cols)


def dedupe_ldweights(nc):
    """Drop InstLdweights whose weights are already resident in the targeted
    PE-array rectangle (Tile emits one LDW per matmul unconditionally).
    Converted to NOPs to preserve semaphore waits/updates. Tracks (row, col)
    rectangles: loads to disjoint row/col groups don't clobber each other."""
    for f in nc.m.functions:
        for bb in f.blocks:
            insts = list(bb.instructions)
            state = {}  # (row_base, col_base) -> (rows, cols, sig)
            changed = False

            def invalidate(rect):
                rb, cb, rn, cn = rect
                for key in list(state):
                    b_rb, b_cb = key
                    b_rn, b_cn = state[key][0], state[key][1]
                    if (
                        b_rb < rb + rn
                        and rb < b_rb + b_rn
                        and b_cb < cb + cn
                        and cb < b_cb + b_cn
                    ):
                        del state[key]

            out = []
            for inst in insts:
                tn = type(inst).__name__
                if tn == "InstLdweights":
                    w = inst.ins[0]
                    rect = _ldw_rect(inst, w)
                    sig = _ldw_sig(
                        str(w), inst.tile_position, inst.perf_mode, inst.is_transpose
                    )
                    key = (rect[0], rect[1])
                    if state.get(key) == (rect[2], rect[3], sig):
                        nop = mybir.InstNoOp(
                            name=f"{inst.name}-ldwdrop", ins=[], outs=[]
                        )
                        nop.engine = inst.engine
                        if inst.sync_info:
                            nop.sync_info = inst.sync_info
                        nc.register_instruction(nop, overwrite=True)
                        out.append(nop)
                        changed = True
                        continue
                    invalidate(rect)
                    state[key] = (rect[2], rect[3], sig)
                elif tn == "InstMatmult":
                    w = inst.ins[-1]
                    rect = _ldw_rect(inst, w)
                    sig = _ldw_sig(
                        str(w), inst.tile_position, inst.perf_mode, inst.is_transpose
                    )
                    key = (rect[0], rect[1])
                    if state.get(key) != (rect[2], rect[3], sig):
                        # self-loading matmul: it loads its own weights
                        invalidate(rect)
                        state[key] = (rect[2], rect[3], sig)
                out.append(inst)
            if changed:
                bb.instructions = out


def apply_tile_patch():
    """Patch TileContext to split >MAX_WAITS sem-waits (incl. final drain),
    and dedupe redundant LDWEIGHTS."""
    import concourse.tile as _tile

    def _patched(self, tick_clock, wait_clock):
        carrier = self.nc.sync.nop(nofuse=True)
        wait_clock.add_sem_waits(
            carrier.ins, _tile.ScopedClock({None: tick_clock.global_clock})
        )
        si = carrier.ins.sync_info
        waits = list(si.on_wait) if si and si.on_wait else []
        if len(waits) > 1:
            carrier.ins.sync_info = mybir.SyncInfo(
                on_wait=waits[:1], on_update=si.on_update
            )
            for w in waits[1:]:
                extra = self.nc.sync.nop(nofuse=True)
                extra.ins.sync_info = mybir.SyncInfo(on_wait=[w], on_update=[])
        self.nc.sync.drain()
        self.nc.all_engine_barrier()
        assert self.sems is not None
        popped = self.nc._tile_sem_poison_stack.pop()
        assert popped is self._sem_poison
        self.nc.clear_and_free_semaphores(list(self.sems.allocated().values()))
        self.nc.all_engine_barrier()
        dedupe_ldweights(self.nc)
        split_excess_waits(self.nc)

    _tile.TileContext._drain_and_barrier = _patched


def build_graph(S=4096, M=2048, DIN=256, DOUT=64, scale=0.125, dve_exp_every=2):
    """One NeuronCore's graph: M queries attend over S keys."""
    assert DIN == 256 and DOUT == 64
    NT = S // 128          # n-tiles (keys)
    NP = NT // 2           # n-tile pairs
    CH = min(512, M)       # query chunk per PSUM bank
    NCH = M // CH          # chunks
    VBLK = 65              # v_sb per-tile block stride (64 v cols + ones col)
    nc = bass.Bass()

    # xT host layout [128, 4, 2, 1024]: per-partition contiguous (c, n) runs
    # so each n-quarter transfer is one 4KB descriptor per partition
    xT_ext = nc.declare_dram_parameter("xT", [128, 4, 2, S // 4], BF16, isOutput=False)
    # wpack: Wq|Wk|Wv ([128, 2, 64] each) + bq2 + bk2 + Rm in one transfer
    NWP = 3 * 128 + 2 + (DOUT + 1)
    wpack_ext = nc.declare_dram_parameter("wpack", [128, NWP], BF16, isOutput=False)
    out_ext = nc.declare_dram_parameter("out", [M, DOUT], F32, isOutput=True)

    # Schraudolph fast-exp in bf16 bit-space (bf16 = top 16 bits of f32):
    # exp(x*scale) ~= bitcast_bf16(i16(A*x + B)), A = 2^7/ln2*scale,
    # B = 127*2^7 - 486411/2^16 (the classic f32 bias scaled down).
    A_C = float((1 << 7) / np.log(2.0) * scale)
    B_C = float(127.0 * (1 << 7) - 486411.0 / 65536.0)

    with tile.TileContext(nc) as tc:
        with (
            tc.tile_pool(name="singles", bufs=1) as singles,
            tc.tile_pool(name="sb_small", bufs=4) as sb_small,
        ):
            # ---- warm memset early: it gates both the PE warmup and the
            # dummy exp that hoists the ACT table load ----
            warm_sb = singles.tile([128, 512], BF16, tag="warm")
            nc.gpsimd.memset(warm_sb, 0.25)
            # dummy exp FIRST on the scalar stream so ACT_TABLE_LOAD runs
            # before (not after) the scalar-issued DMA descriptor-gens
            dum = sb_small.tile([128, 1], F32, tag="r", name="dummy")
            nc.scalar.activation(dum, warm_sb[:, 0:1], AF.Exp, scale=scale)

            # ---- input DMA: one packed small-tensor transfer + 4 x-quarter
            # transfers (dma_start costs ~650ns of sequencer time each; the
            # scalar sequencer is also the ACT engine, so keep its list short)
            wpack_sb = singles.tile([128, NWP], BF16, tag="wpack")
            nc.scalar.dma_start(out=wpack_sb, in_=wpack_ext[:])
            w_sb = {
                w: wpack_sb[:, 128 * i : 128 * i + 128].rearrange(
                    "p (c d) -> p c d", c=2
                )
                for i, w in enumerate(("Wq", "Wk", "Wv"))
            }
            b_sb = {
                "bq2": wpack_sb[:, 384:385],
                "bk2": wpack_sb[:, 385:386],
            }
            # R matrix for the bias-folding epilogue transpose, built on host:
            # R[0:64,0:64] = I, R[64,0:64] = bv, R[64,64] = 1, R[0:64,64] = 0
            Rm = wpack_sb[0:VBLK, 386 : 386 + VBLK]

            xT_sb = singles.tile([128, 2, S], BF16)
            # three rings pull x: sync (2 quarters), scalar (1, after wpack),
            # gpsimd SWDGE (1) — keeps the scalar/ACT sequencer list short
            NDC = 4
            for dchunk, eng in enumerate((nc.sync, nc.gpsimd, nc.scalar, nc.sync)):
                n0 = dchunk * (S // NDC)
                eng.dma_start(
                    out=xT_sb[:, :, n0 : n0 + S // NDC],
                    in_=xT_ext[:, dchunk, :, :],
                )

            prev_pe = [None]

            def chain(bi):
                # serialize PE matmuls in emission order so same-weights runs
                # stay adjacent (LDW dedup) and pipelining is stable
                if prev_pe[0] is not None:
                    tile.add_dep_helper(
                        bi.ins, prev_pe[0].ins, sync=False, reason="pe-order"
                    )
                prev_pe[0] = bi

            qT2_sb = singles.tile([128, M], BF16, tag="qT2")
            kT2_sb = singles.tile([128, S // 2], BF16, tag="kT2")
            v_sb = singles.tile([128, NT * VBLK], BF16, tag="vsb")
            # only the ones columns of v_sb need initializing (col 64 of
            # each VBLK block); the v copies fill cols 0:64
            nc.gpsimd.memset(
                v_sb.rearrange("p (b r) -> p b r", r=VBLK)[:, :, 64:65], 1.0
            )

            PQ = min(512, M)           # q quarter cols
            KQ = min(512, S // 2)      # k quarter cols (packed)
            TPQ = KQ // 128            # n-tile pairs per k quarter
            NKQ = (S // 2) // KQ

            # ---- projections (before attention, own pools: 3 bufs each plus
            # warm inside ppool so only 7 banks are held; spool's first bufs
            # then land on early-freed banks) ----
            ppool_cm = tc.tile_pool(name="ppsum", bufs=3, space="PSUM")
            ppool = ppool_cm.__enter__()
            vpool_cm = tc.tile_pool(name="vpsum", bufs=3, space="PSUM")
            vpool = vpool_cm.__enter__()

            wps = ppool.tile([128, 512], F32, tag="proj", name="warm")
            for _ in range(4):
                chain(
                    nc.tensor.matmul(
                        wps, lhsT=warm_sb[:, 0:128], rhs=warm_sb[:, 0:512],
                        start=True, stop=True,
                    )
                )

            def pemit_proj_q(qi):
                ps = ppool.tile([128, PQ], F32, tag="proj", name=f"psq_{qi}")
                for cg in range(2):
                    for c in range(2):
                        nc.tensor.matmul(
                            ps[64 * cg : 64 * cg + 64, :],
                            lhsT=w_sb["Wq"][:, c, :],
                            rhs=xT_sb[:, c, PQ * qi : PQ * qi + PQ],
                            start=(c == 0),
                            stop=(c == 1),
                            tile_position=(0, 64 * cg),
                        )
                nc.scalar.add(qT2_sb[:, PQ * qi : PQ * qi + PQ], ps, b_sb["bq2"])

            def pemit_proj_k(qi):
                ps = ppool.tile([128, KQ], F32, tag="proj", name=f"psk_{qi}")
                for cg in range(2):
                    for c in range(2):
                        xv = xT_sb[:, c, :].rearrange(
                            "p (u two j) -> p u two j", two=2, j=128
                        )
                        nc.tensor.matmul(
                            ps[64 * cg : 64 * cg + 64, :],
                            lhsT=w_sb["Wk"][:, c, :],
                            rhs=xv[:, TPQ * qi : TPQ * qi + TPQ, cg, :],
                            start=(c == 0),
                            stop=(c == 1),
                            tile_position=(0, 64 * cg),
                        )
                nc.scalar.add(kT2_sb[:, KQ * qi : KQ * qi + KQ], ps, b_sb["bk2"])

            def pemit_proj_v(g):
                ps = vpool.tile([128, 4, DOUT], F32, tag="vnat", name=f"psv_{g}")
                for i in range(4):
                    nt = 4 * g + i
                    for c in range(2):
                        nc.tensor.matmul(
                            ps[:, i, :],
                            lhsT=xT_sb[:, c, 128 * nt : 128 * nt + 128],
                            rhs=w_sb["Wv"][:, c, :],
                            start=(c == 0),
                            stop=(c == 1),
                        )
                dst = v_sb.rearrange("p (b r) -> p b r", r=VBLK)[
                    :, 4 * g : 4 * g + 4, 0:64
                ]
                nc.vector.tensor_copy(dst, ps)

            pemit_proj_q(0)
            for qi in range(NKQ):
                pemit_proj_k(qi)
                for g in range(2 * qi, 2 * qi + 2):
                    pemit_proj_v(g)
            vpool_cm.__exit__(None, None, None)
            ppool_cm.__exit__(None, None, None)

            # ---- attention: chunk-outer, pair-inner; exp alternates ACT/DVE;
            # q quarters 1-3 are emitted inside chunk 0's pair stream.
            attn_psum = ExitStack()
            spool = attn_psum.enter_context(
                tc.tile_pool(name="spsum", bufs=3, space="PSUM")
            )
            opool = attn_psum.enter_context(
                tc.tile_pool(
                    name="opsum", bufs=(2 if AV_SPLIT else 1), space="PSUM"
                )
            )
            ptpool = (
                None
                if AV_SPLIT
                else attn_psum.enter_context(
                    tc.tile_pool(name="ptpsum", bufs=1, space="PSUM")
                )
            )
            with (
                attn_psum,
                tc.tile_pool(name="pexp", bufs=3) as ppexp,
                tc.tile_pool(name="oout", bufs=2) as oout,
            ):
                pending_steps = []

                def emit_proj_q_late(qi):
                    # q quarters 1-3 are only needed from chunk 1 on; emit
                    # them inside chunk 0's pair stream, borrowing an s-ring
                    # slot for the PSUM (ppool is closed by now).
                    ps = spool.tile(
                        [128, 2 * CH], F32, tag="s", name=f"qps_{qi}"
                    )
                    for cg in range(2):
                        for c in range(2):
                            nc.tensor.matmul(
                                ps[64 * cg : 64 * cg + 64, 0:PQ],
                                lhsT=w_sb["Wq"][:, c, :],
                                rhs=xT_sb[:, c, PQ * qi : PQ * qi + PQ],
                                start=(c == 0),
                                stop=(c == 1),
                                tile_position=(0, 64 * cg),
                            )
                    nc.scalar.add(
                        qT2_sb[:, PQ * qi : PQ * qi + PQ], ps[:, 0:PQ],
                        b_sb["bq2"],
                    )

                proj_work = [
                    (lambda qi=qi: emit_proj_q_late(qi))
                    for qi in range(1, M // PQ)
                ]
                for mc in range(NCH):
                    mlo = CH * mc
                    po = opool.tile([VBLK, CH], F32, tag="po", name=f"po_{mc}")
                    po_b = (
                        opool.tile([VBLK, CH], F32, tag="po", name=f"pob_{mc}")
                        if AV_SPLIT
                        else None
                    )
                    s_t = [None] * NP
                    p_t = [None] * NP

                    def emit_qk_exp(j):
                        s = spool.tile(
                            [128, 2 * CH], F32, tag="s", name=f"s_{mc}_{j}"
                        )
                        for half in range(2):
                            nc.tensor.matmul(
                                s[:, CH * half : CH * half + CH],
                                lhsT=kT2_sb[
                                    64 * half : 64 * half + 64,
                                    128 * j : 128 * j + 128,
                                ],
                                rhs=qT2_sb[
                                    64 * half : 64 * half + 64, mlo : mlo + CH
                                ],
                                start=True,
                                stop=True,
                                tile_position=(64 * half, 0),
                            )
                        s_t[j] = s
                        di = mc * NP + j
                        if dve_exp_every and di % 16 in (1, 3, 5, 7, 9, 11, 13):
                            # Schraudolph fast-exp on the (otherwise idle) DVE;
                            # AV reads the bf16 bitcast directly (no copy).
                            i16 = ppexp.tile(
                                [128, 2 * CH], mybir.dt.int16, tag="pi",
                                name=f"pi_{mc}_{j}",
                            )
                            nc.vector.tensor_scalar(
                                i16, s, A_C, B_C,
                                op0=mybir.AluOpType.mult,
                                op1=mybir.AluOpType.add,
                            )
                            p_t[j] = i16.bitcast(BF16)
                        else:
                            p = ppexp.tile(
                                [128, 2 * CH], BF16, tag="p", name=f"p_{mc}_{j}"
                            )
                            nc.scalar.activation(p, s, AF.Exp, scale=scale)
                            p_t[j] = p

                    def emit_av(j):
                        # AV_SPLIT: 2-way K-split (rows 0:64 / 64:128) per
                        # n-tile into separate po banks: the two row-tiled
                        # matmuls stream concurrently and each half's
                        # LDWEIGHTS overlaps the other half's matmul
                        for half in range(2):
                            vt = v_sb[
                                :,
                                VBLK * (2 * j + half) : VBLK * (2 * j + half)
                                + VBLK,
                            ]
                            pt_ = p_t[j][:, CH * half : CH * half + CH]
                            if AV_SPLIT:
                                for ks, dst in ((0, po), (1, po_b)):
                                    nc.tensor.matmul(
                                        dst,
                                        lhsT=vt[64 * ks : 64 * ks + 64, :],
                                        rhs=pt_[64 * ks : 64 * ks + 64, :],
                                        start=(j == 0 and half == 0),
                                        stop=(j == NP - 1 and half == 1),
                                        tile_position=(64 * ks, 0),
                                    )
                            else:
                                nc.tensor.matmul(
                                    po,
                                    lhsT=vt,
                                    rhs=pt_,
                                    start=(j == 0 and half == 0),
                                    stop=(j == NP - 1 and half == 1),
                                )
                        s_t[j] = None
                        p_t[j] = None

                    emit_qk_exp(0)
                    emit_qk_exp(1)
                    for j in range(NP):
                        for _ in range(2):
                            if proj_work:
                                proj_work.pop(0)()
                        if pending_steps:
                            pending_steps.pop(0)()
                        if j + 2 < NP:
                            emit_qk_exp(j + 2)
                        emit_av(j)

                    # epilogue: matmul-by-R (adds bv), divide by l, store.
                    # Emitted as one step per pair of the NEXT chunk so the PE
                    # keeps streaming QKs across the chunk boundary and the
                    # single-buffer pt ring never stalls it. bf16 operands:
                    # plain matmul (transpose-mode ignores rhs content, so R
                    # must go through the regular path) and bf16 avoids the
                    # slow fp32 LOW/HIGH double-pass.
                    cell = {}

                    def step_obcopy(mc=mc, po=po, po_b=po_b):
                        # only one PSUM input allowed per instruction: ACT
                        # narrows po_a to bf16, DVE adds po_b on top
                        ob = oout.tile([VBLK, CH], BF16, tag="ob", name=f"ob_{mc}")
                        if AV_SPLIT:
                            ob_a = oout.tile(
                                [VBLK, CH], BF16, tag="oba", name=f"oba_{mc}"
                            )
                            nc.scalar.copy(ob_a, po)
                            nc.vector.tensor_add(ob, ob_a, po_b)
                        else:
                            nc.scalar.copy(ob, po)
                        cell["ob"] = ob
                        cell["zb"] = oout.tile(
                            [128, CH // 128, 64], F32, tag="zb", name=f"zb_{mc}"
                        )

                    def step_t(t, mc=mc):
                        # strided ob slice: pt partition p = query mlo+4p+t,
                        # so each out-DMA descriptor covers 4 contiguous rows
                        ob, zb = cell["ob"], cell["zb"]
                        # under AV_SPLIT the pt transposes borrow s-ring
                        # slots (brief holds); this also pipelines the final
                        # chunk's epilogue when the ring is free
                        if AV_SPLIT or mc == NCH - 1:
                            pt = spool.tile(
                                [128, 2 * CH], F32, tag="s", name=f"zt_{mc}_{t}"
                            )[:, 0:VBLK]
                        else:
                            pt = ptpool.tile(
                                [128, VBLK], F32, tag="pt", name=f"zt_{mc}_{t}"
                            )
                        obv = ob.rearrange("v (f four) -> v f four", four=4)
                        nc.tensor.matmul(
                            pt,
                            lhsT=obv[:, :, t],
                            rhs=Rm,
                            start=True,
                            stop=True,
                        )
                        r = sb_small.tile([128, 1], F32, tag="r", name="r_t")
                        nc.vector.reciprocal(r, pt[:, 64:65])
                        nc.vector.tensor_scalar_mul(zb[:, t, :], pt[:, 0:64], r)

                    def step_dma(mlo=mlo):
                        nc.sync.dma_start(
                            out=out_ext[mlo : mlo + CH, :].rearrange(
                                "(p four) d -> p four d", four=4
                            ),
                            in_=cell.pop("zb"),
                        )
                        cell.pop("ob")

                    pending_steps = [step_obcopy]
                    pending_steps += [
                        (lambda t=t: step_t(t)) for t in range(CH // 128)
                    ]
                    pending_steps.append(step_dma)
                for st in pending_steps:
                    st()
                pending_steps = []
    return nc


def _make_rmat(bv):
    import ml_dtypes

    d = bv.shape[0]
    R = np.zeros((d + 1, d + 1), np.float32)
    R[:d, :d] = np.eye(d, dtype=np.float32)
    R[d, :d] = bv
    R[d, d] = 1.0
    return R.astype(ml_dtypes.bfloat16)


def make_in_maps(x, Wq, bq, Wk, bk, Wv, bv, n_cores=8):
    """Host-side sharding: core i handles batch i//2, query half i%2."""
    import ml_dtypes

    bf16 = ml_dtypes.bfloat16
    B, S, DIN = x.shape
    M = S // 2
    DOUT = Wq.shape[1]

    def wrepack(W):
        # [DIN, DOUT] -> [128, 2*DOUT] with c-tile-major per partition
        return (
            np.ascontiguousarray(W)
            .reshape(2, 128, DOUT)
            .transpose(1, 0, 2)
            .reshape(128, 2 * DOUT)
        )

    rm = np.zeros((128, DOUT + 1), np.float64)
    rm[: DOUT + 1] = _make_rmat(bv).astype(np.float64)
    wpack = np.concatenate(
        [
            wrepack(Wq),
            wrepack(Wk),
            wrepack(Wv),
            np.concatenate([bq, bq]).reshape(128, 1),
            np.concatenate([bk, bk]).reshape(128, 1),
            rm,
        ],
        axis=1,
    ).astype(bf16)

    in_maps = []
    for i in range(n_cores):
        b, half = i // 2, i % 2
        xb = np.roll(x[b], -half * M, axis=0)  # own queries first
        # [128, 4, 2, S/4]: per-partition contiguous (c, n) quarter runs
        xT = np.ascontiguousarray(
            xb.reshape(4, S // 4, 2, 128).transpose(3, 0, 2, 1)
        ).astype(bf16)
        in_maps.append({"xT": xT, "wpack": wpack})
    return in_maps


def assemble_out(results, B=4, S=4096, DOUT=64):
    M = S // 2
    z = np.empty((B, S, DOUT), np.float32)
    for i, res in enumerate(results):
        b, half = i // 2, i % 2
        z[b, half * M : (half + 1) * M] = res["out"]
    return z


_GRAPH_CACHE = {}


def kernel(x, Wq, bq, Wk, bk, Wv, bv):
    """Full-input entry point: shards across 8 NeuronCores (batch x
    query-half), runs the Bass kernel SPMD, gathers the full [B, S, 64]
    float32 output."""
    from concourse.bass_utils import run_bass_kernel_spmd

    apply_tile_patch()
    x = np.asarray(x, dtype=np.float32)
    Wq, bq = np.asarray(Wq, np.float32), np.asarray(bq, np.float32)
    Wk, bk = np.asarray(Wk, np.float32), np.asarray(bk, np.float32)
    Wv, bv = np.asarray(Wv, np.float32), np.asarray(bv, np.float32)
    B, S, DIN = x.shape
    DOUT = Wq.shape[1]
    key = (S, DIN, DOUT)
    if key not in _GRAPH_CACHE:
        _GRAPH_CACHE[key] = build_graph(
            S=S, M=S // 2, DIN=DIN, DOUT=DOUT, scale=1.0 / float(np.sqrt(DOUT))
        )
    nc = _GRAPH_CACHE[key]
    in_maps = make_in_maps(x, Wq, bq, Wk, bk, Wv, bv, n_cores=2 * B)
    res = run_bass_kernel_spmd(nc, in_maps, list(range(2 * B)))
    return assemble_out(res.results, B=B, S=S, DOUT=DOUT)

